# revision 19
# baseline (speedup 1.0000x reference)
"""Tropical (max-plus) 3x3 conv kernel for Trainium2, batch-parallel over 8 cores.

Problem: imgs [8,32,32,32] f32, kernel [32,32,3,3] f32, padding=1 with -inf,
conv-style spatial flip, out[b,o,y,x] = max_{c,dy,dx}(imgs_pad[b,c,y+dy,x+dx]
+ kernel[o,c,2-dy,2-dx]).  Output [8,32,32,32] f32.

Host prep (sharding/layout): per-core batch slice is pre-padded with -inf to
[32, 34*34] so the device DMA is contiguous and needs no memset; the kernel
tensor is pre-arranged to [(o4 c), (g t)] = [128, 72] with the spatial flip
applied by tap indexing on device; the PE-transpose identity ships from host.

Per-core device program (1 batch element per core):
  partitions p = (o4, c): 4 output channels x 32 input channels; padded image
  replicated across the 4 o4-blocks by 4 DMA reads of the same DRAM source,
  spread across engine DMA queues.  For each of 8 o-groups, a chain of fused
  scalar_tensor_tensor ops computes acc = max(acc, window_t + k[o,c,t]) over
  the 9 taps (first tap via 2x-mode tensor_scalar).  Channel reduction: PE
  transpose (128x128 chunks) to PSUM, one segmented tensor_reduce(max) per
  group, second PE transpose to [o, yx] layout, ScalarE copy to SBUF, DMA out.
"""

import numpy as np

import concourse.bacc as bacc
import concourse.mybir as mybir
import concourse.tile as tile
from concourse.bass_utils import run_bass_kernel_spmd

B, C, H, W = 8, 32, 32, 32
O, KH, KW = 32, 3, 3
PAD = 1
PH, PW = H + 2 * PAD, W + 2 * PAD  # 34, 34
OY, OX = H, W  # 32, 32 (stride 1, 3x3, pad 1)
N_CORES = 8
F32 = mybir.dt.float32
NEG_INF = float("-inf")


def build():
    nc = bacc.Bacc(
        "TRN2",
        target_bir_lowering=False,
        debug=False,
        num_devices=N_CORES,
    )
    padimg = nc.dram_tensor("padimg", [128, PH * PW], F32, kind="ExternalInput")
    ktab = nc.dram_tensor("ktab", [128, 8 * 9], F32, kind="ExternalInput")
    idin = nc.dram_tensor("idin", [128, 128], F32, kind="ExternalInput")
    out = nc.dram_tensor("out", [O, OY, OX], F32, kind="ExternalOutput")

    add = mybir.AluOpType.add
    vmax = mybir.AluOpType.max

    with tile.TileContext(nc) as tc:
        with (
            tc.tile_pool(name="const", bufs=1) as cpool,
            tc.tile_pool(name="accp", bufs=2) as apool,
            tc.tile_pool(name="redp", bufs=2) as rpool,
            tc.tile_pool(name="psp", bufs=2, space="PSUM") as pspool,
            tc.tile_pool(name="ps2p", bufs=2, space="PSUM") as ps2pool,
        ):
            pad = cpool.tile([128, PH * PW], F32)
            ktile = cpool.tile([128, 8 * 9], F32)
            ident = cpool.tile([128, 128], F32)

            # padded image arrives pre-replicated across the 4 o4-blocks, so
            # full-width (128-partition) DMAs load it at full SBUF BW; split
            # the free dim across two queues to halve transfer latency
            half = (PH * PW) // 2
            nc.sync.dma_start(out=pad[:, :half], in_=padimg.ap()[:, :half])
            nc.sync.dma_start(out=pad[:, half:], in_=padimg.ap()[:, half:])
            nc.gpsimd.dma_start(out=ktile[:], in_=ktab.ap())
            nc.gpsimd.dma_start(out=ident[:], in_=idin.ap())

            pad3 = pad[:].rearrange("p (y x) -> p y x", y=PH)
            # out[o,y,x] viewed as [g, (a ck), (fy x)]: o = g*4+a, yx = ck*128+fy*32+x
            outv = out.ap().rearrange("(g a) (ck fy) x -> g (a ck) (fy x)", a=4, fy=4)

            for g in range(8):
                acc = apool.tile([128, OY * OX], F32, tag="acc")
                acc3 = acc[:].rearrange("p (y x) -> p y x", y=OY)
                for t in range(9):
                    dy, dx = divmod(t, 3)
                    win = pad3[:, dy : dy + OY, dx : dx + OX]
                    # spatial flip: window shift (dy,dx) uses kernel tap (2-dy,2-dx)
                    sc = ktile[:, g * 9 + (8 - t) : g * 9 + (8 - t) + 1]
                    if t == 0:
                        nc.vector.tensor_scalar_add(acc3, win, sc)
                    else:
                        nc.vector.scalar_tensor_tensor(acc3, win, sc, acc3, add, vmax)

                ps = pspool.tile([128, OY * OX], F32, tag="ps")
                for ck in range(8):
                    nc.tensor.transpose(
                        ps[:, ck * 128 : (ck + 1) * 128],
                        acc[:, ck * 128 : (ck + 1) * 128],
                        ident[:],
                    )
                # transposed: partition = yx_local, free = (ck, a, c); reduce over c
                ps4 = ps[:].rearrange("p (ck a c) -> p a ck c", ck=8, a=4)
                red = rpool.tile([128, 32], F32, tag="red")
                red3 = red[:].rearrange("p (a ck) -> p a ck", a=4)
                nc.vector.tensor_reduce(
                    red3, ps4, axis=mybir.AxisListType.X, op=vmax
                )
                ps2 = ps2pool.tile([32, 128], F32, tag="ps2")
                nc.tensor.transpose(ps2[:], red[:], ident[:])
                osb = rpool.tile([32, 128], F32, tag="osb")
                nc.scalar.copy(osb[:], ps2[:])
                nc.sync.dma_start(out=outv[g], in_=osb[:])

    nc.compile()
    return nc


_NC_CACHE = None


def _get_nc():
    global _NC_CACHE
    if _NC_CACHE is None:
        _NC_CACHE = build()
    return _NC_CACHE


def make_in_maps(imgs, kernel):
    imgs = np.ascontiguousarray(np.asarray(imgs), dtype=np.float32)
    kern = np.ascontiguousarray(np.asarray(kernel), dtype=np.float32)
    assert imgs.shape == (B, C, H, W) and kern.shape == (O, C, KH, KW)
    # [(o4 c), (g t)]: ktab[a*32+c, g*9+t] = kern[g*4+a, c, dy, dx], t = dy*3+dx
    ktab = np.ascontiguousarray(
        kern.reshape(8, 4, C, 9).transpose(1, 2, 0, 3).reshape(128, 72)
    )
    padded = np.full((B, C, PH, PW), NEG_INF, dtype=np.float32)
    padded[:, :, PAD : PAD + H, PAD : PAD + W] = imgs
    padded = padded.reshape(B, C, PH * PW)
    ident = np.eye(128, dtype=np.float32)
    return [
        {
            "padimg": np.ascontiguousarray(np.tile(padded[i], (4, 1))),
            "ktab": ktab,
            "idin": ident,
        }
        for i in range(N_CORES)
    ]


def assemble(results):
    return np.stack([np.asarray(r["out"]) for r in results], axis=0)


def kernel(imgs, kernel):
    nc = _get_nc()
    res = run_bass_kernel_spmd(nc, make_in_maps(imgs, kernel), list(range(N_CORES)))
    return assemble(res.results)
